# revision 28
# baseline (speedup 1.0000x reference)
"""Trainium2 Bass kernel for nn_BlockMoE (8-core SPMD).

Sharding: data-parallel attention (1 batch element per core), expert-parallel
MoE (1 expert per core) with AllGather token dispatch + indirect-DMA
gather/scatter and ReduceScatter combine. Matmuls run in float32r (fast fp32).

Self-contained: hardcodes shapes B=8, T=197, D=768, E=8, H=3072, heads=12.
"""
import numpy as np

import concourse.bass as bass
import concourse.bacc as bacc
import concourse.mybir as mybir
import concourse.tile as tile
from concourse.masks import make_identity

dt = mybir.dt
AF = mybir.ActivationFunctionType
OP = mybir.AluOpType

# problem shapes
B, T, D = 8, 197, 768
E, H = 8, 3072
HEADS, HD = 12, 64
SCALE = HD ** -0.5
EPS = 1e-5
BALANCE_COEF, ROUTER_Z_COEF = 1e-2, 1e-3

NCORES = 8
CORE_IDS = list(range(NCORES))
P = 128
KC = D // P                      # 6 d-chunks
N = B * T                        # 1576 tokens
JN = 13                          # token chunks of 128 (padded)
NPAD = JN * P                    # 1664
TB = T                           # rows shipped per core
ZROW = NPAD                      # zero row index in all_data
ROWS = NPAD + 1                  # 1665
CAP = 512                        # expert capacity (~394 expected)
G = CAP // P                     # 4 gather tiles
TP = 256                         # padded token free-dim per core
TROWS = (P, T - P)               # (128, 69) token partition chunks
HT = H // P                      # 24

# all_data row layout
C_XN2, C_CMB, C_IND, C_PRB, C_Z2, WIDE = 0, 768, 776, 784, 792, 800

f32, f32r, i32 = dt.float32, dt.float32r, dt.int32
X = mybir.AxisListType.X


def _ln(nc, pool, xt, rows, wrow, brow, out, eps_ap=None):
    """LayerNorm of xt[:rows, :D] -> out[:rows, :D]; wrow/brow [128,D] replicated."""
    s = pool.tile([P, 1], f32, tag="ln_s", name="ln_s")
    nc.vector.tensor_reduce(s[:rows], xt[:rows], axis=X, op=OP.add)
    negmu = pool.tile([P, 1], f32, tag="ln_negmu", name="ln_negmu")
    nc.vector.tensor_scalar_mul(negmu[:rows], s[:rows], -1.0 / D)
    xc = pool.tile([P, D], f32, tag="ln_xc", name="ln_xc")
    nc.vector.tensor_scalar_add(xc[:rows], xt[:rows], negmu[:rows])
    sq = pool.tile([P, D], f32, tag="ln_sq", name="ln_sq")
    var = pool.tile([P, 1], f32, tag="ln_var", name="ln_var")
    nc.scalar.activation(sq[:rows], xc[:rows], AF.Square, accum_out=var[:rows])
    std = pool.tile([P, 1], f32, tag="ln_std", name="ln_std")
    nc.scalar.activation(std[:rows], var[:rows], AF.Sqrt, bias=eps_ap[:rows],
                         scale=1.0 / D)
    rstd = pool.tile([P, 1], f32, tag="ln_rstd", name="ln_rstd")
    nc.vector.reciprocal(rstd[:rows], std[:rows])
    nc.vector.tensor_scalar_mul(xc[:rows], xc[:rows], rstd[:rows])
    nc.vector.tensor_tensor(out[:rows], xc[:rows], wrow[:rows], op=OP.mult)
    nc.vector.tensor_tensor(out[:rows], out[:rows], brow[:rows], op=OP.add)


def build_nc():
    nc = bacc.Bacc("TRN2", target_bir_lowering=False, debug=False,
                   num_devices=NCORES)
    MM = nc.tensor.matmul

    # ---- per-core external I/O ----
    x_d = nc.dram_tensor("x", [T, D], f32, kind="ExternalInput")
    wqkv_d = nc.dram_tensor("Wqkv", [D, 3 * D], f32r, kind="ExternalInput")
    wproj_d = nc.dram_tensor("Wproj", [D, D], f32r, kind="ExternalInput")
    wg_d = nc.dram_tensor("Wg", [D, E], f32, kind="ExternalInput")
    n1w_d = nc.dram_tensor("norm1_w", [D], f32, kind="ExternalInput")
    n1b_d = nc.dram_tensor("norm1_b", [D], f32, kind="ExternalInput")
    n2w_d = nc.dram_tensor("norm2_w", [D], f32, kind="ExternalInput")
    n2b_d = nc.dram_tensor("norm2_b", [D], f32, kind="ExternalInput")
    w1_d = nc.dram_tensor("W1", [D, H], f32r, kind="ExternalInput")
    b1_d = nc.dram_tensor("b1", [H], f32, kind="ExternalInput")
    w2_d = nc.dram_tensor("W2", [H, D], f32r, kind="ExternalInput")
    b2_d = nc.dram_tensor("b2", [D], f32, kind="ExternalInput")
    esel_d = nc.dram_tensor("expert_sel", [1, E], f32, kind="ExternalInput")

    out_d = nc.dram_tensor("out", [T, D], f32, kind="ExternalOutput")
    bal_d = nc.dram_tensor("balance", [1, 1], f32, kind="ExternalOutput")
    rz_d = nc.dram_tensor("router_z", [1, 1], f32, kind="ExternalOutput")
    tot_d = nc.dram_tensor("total_aux", [1, 1], f32, kind="ExternalOutput")

    with tile.TileContext(nc) as tc:
        dram = tc.alloc_tile_pool(name="dram", bufs=1, space="DRAM")
        ag_src = dram.tile([TB, WIDE], f32, tag="ag_src", name="ag_src")
        all_data = dram.tile([ROWS, WIDE], f32, tag="all_data", name="all_data")
        gl = dram.tile([CAP + 1, 1], i32, tag="gl", name="gl")
        moe_part = dram.tile([ROWS, D], f32, tag="moe_part", name="moe_part")
        rs_res = dram.tile([TB, D], f32, tag="rs_res", name="rs_res")

        const = tc.alloc_tile_pool(name="const", bufs=1)

        def ct(shape, dtype, tg):
            return const.tile(shape, dtype, tag=tg, name=tg)

        ident = ct([P, P], f32, "ident")
        make_identity(nc, ident)
        ones_f = ct([P, 1], f32, "ones_f")
        nc.vector.memset(ones_f[:], 1.0)
        ones_r = ct([P, 1], f32r, "ones_r")
        nc.vector.tensor_copy(ones_r[:], ones_f[:])
        one_row = ct([1, P], f32, "one_row")
        nc.vector.memset(one_row[:], 1.0)
        LT = ct([P, P], f32, "LT")  # LT[p, m] = 1 if p < m
        nc.gpsimd.memset(LT[:], 1.0)
        nc.gpsimd.affine_select(out=LT[:], in_=LT[:], compare_op=OP.is_gt,
                                fill=0.0, base=0, pattern=[[1, P]],
                                channel_multiplier=-1)
        zrow = ct([P, WIDE], f32, "zrow")
        nc.vector.memset(zrow[:], 0.0)
        eps_t = ct([P, 1], f32, "eps_t")
        nc.vector.memset(eps_t[:], EPS)
        # padbias[p] = 0 for p < 69 (valid tail tokens), -1e30 beyond -> exp()=0
        padbias = ct([P, 1], f32, "padbias")
        nc.gpsimd.memset(padbias[:], 0.0)
        nc.gpsimd.affine_select(out=padbias[:], in_=padbias[:],
                                compare_op=OP.is_ge, fill=-1e30,
                                base=TROWS[1] - 1, pattern=[[1, 1]],
                                channel_multiplier=-1)

        nrm_bc = ct([P, 4 * D], f32, "nrm_bc")
        b2_bc = ct([P, D], f32, "b2_bc")
        esel_bc = ct([P, E], f32, "esel_bc")
        b1_sb = ct([P, HT], f32, "b1_sb")
        nc.sync.dma_start(b1_sb[:], b1_d.ap().rearrange("(o p) -> p o", p=P))

        # broadcast rows via K=1 matmul
        with tc.tile_pool(name="bcld", bufs=1) as blp, \
             tc.tile_pool(name="bcps", bufs=2, space="PSUM") as bpp:
            nrm = blp.tile([1, 4 * D], f32, tag="nrm", name="nrm")
            nc.sync.dma_start(nrm[:1, 0:D], n1w_d.ap()[None, :])
            nc.sync.dma_start(nrm[:1, D:2 * D], n1b_d.ap()[None, :])
            nc.sync.dma_start(nrm[:1, 2 * D:3 * D], n2w_d.ap()[None, :])
            nc.sync.dma_start(nrm[:1, 3 * D:4 * D], n2b_d.ap()[None, :])
            b2r = blp.tile([1, D], f32, tag="b2r", name="b2r")
            nc.sync.dma_start(b2r[:1, :], b2_d.ap()[None, :])
            eselr = blp.tile([1, E], f32, tag="eselr", name="eselr")
            nc.sync.dma_start(eselr[:1, :], esel_d.ap()[:1, :])
            for src, dst, width in ((nrm, nrm_bc, 4 * D), (b2r, b2_bc, D),
                                    (eselr, esel_bc, E)):
                done = 0
                while done < width:
                    w = min(512, width - done)
                    pt = bpp.tile([P, 512], f32, tag="bc", name="bc_ps")
                    MM(pt[:, :w], one_row[:1], src[:1, done:done + w],
                       start=True, stop=True)
                    nc.vector.tensor_copy(dst[:, done:done + w], pt[:, :w])
                    done += w

        n1w, n1b = nrm_bc[:, 0:D], nrm_bc[:, D:2 * D]
        n2w, n2b = nrm_bc[:, 2 * D:3 * D], nrm_bc[:, 3 * D:4 * D]

        # zero pad rows of all_data (rows N..ROWS) and all of moe_part (early)
        nc.sync.dma_start(all_data[N:ROWS], zrow[:ROWS - N])
        for j in range(JN):
            nc.sync.dma_start(moe_part[j * P:(j + 1) * P], zrow[:, :D])
        nc.sync.dma_start(moe_part[NPAD:ROWS], zrow[:1, :D])


        # ================= attention (own batch) =================
        with tc.tile_pool(name="ap1", bufs=1) as a1, \
             tc.tile_pool(name="aw", bufs=2) as aw, \
             tc.tile_pool(name="psA", bufs=1, space="PSUM") as psA:

            def at1(shape, dtype, tg):
                return a1.tile(shape, dtype, tag=tg, name=tg)

            def psum(tg, bufs, shape=(P, 512)):
                return psA.tile(list(shape), f32, tag=tg, name=tg, bufs=bufs)

            xt, xn = [], []
            for ti, rows in enumerate(TROWS):
                xi = at1([P, D], f32, f"x{ti}")
                nc.sync.dma_start(xi[:rows], x_d.ap()[ti * P:ti * P + rows])
                xt.append(xi)
                xni = at1([P, D], f32, f"xn{ti}")
                _ln(nc, aw, xi, rows, n1w, n1b, xni, eps_t)
                xn.append(xni)

            # xnT [6][128, TP] f32r, zero-padded cols
            xnT = []
            for kc in range(KC):
                xTk = at1([P, TP], f32r, f"xnT{kc}")
                nc.vector.tensor_copy(xTk[:, T:], zrow[:, :TP - T])
                for ti, rows in enumerate(TROWS):
                    pt = psum("tr", 2, (P, P))
                    nc.tensor.transpose(pt[:, :rows],
                                        xn[ti][:rows, kc * P:(kc + 1) * P],
                                        ident[:rows, :rows])
                    nc.vector.tensor_copy(xTk[:, ti * P:ti * P + rows], pt[:, :rows])
                xnT.append(xTk)

            wqkv = []
            for kc in range(KC):
                wk = at1([P, 3 * D], f32r, f"wqkv{kc}")
                nc.sync.dma_start(wk[:], wqkv_d.ap()[kc * P:(kc + 1) * P])
                wqkv.append(wk)

            qT, kT = [], []
            for jt in range(12):
                pt = psum("mm", 4, (P, TP))
                for kc in range(KC):
                    MM(pt[:], wqkv[kc][:, jt * P:(jt + 1) * P], xnT[kc][:],
                       start=(kc == 0), stop=(kc == KC - 1))
                o = at1([P, TP], f32r, f"qkT{jt}")
                if jt < 6:
                    nc.vector.tensor_scalar_mul(o[:], pt[:], SCALE)
                    qT.append(o)
                else:
                    nc.vector.tensor_copy(o[:], pt[:])
                    kT.append(o)

            v = []
            for ti in range(2):
                vt = at1([P, D], f32r, f"v{ti}")
                for nf0, nfw in ((0, 512), (512, 256)):
                    pt = psum("mm", 4)
                    for kc in range(KC):
                        MM(pt[:, :nfw], xnT[kc][:, ti * P:(ti + 1) * P],
                           wqkv[kc][:, 2 * D + nf0:2 * D + nf0 + nfw],
                           start=(kc == 0), stop=(kc == KC - 1))
                    nc.vector.tensor_copy(vt[:, nf0:nf0 + nfw], pt[:, :nfw])
                v.append(vt)

            aT = []
            for h in range(HEADS):
                jk, po = h // 2, (h % 2) * HD
                expT = []
                for jc in range(2):
                    ps = psum("mm", 4, (P, TP))
                    MM(ps[:], kT[jk][po:po + HD, jc * P:(jc + 1) * P],
                       qT[jk][po:po + HD, :], start=True, stop=True)
                    ex = aw.tile([P, TP], f32r, tag=f"expT{jc}", name=f"expT{jc}")
                    if jc == 1:
                        nc.scalar.activation(ex[:], ps[:], AF.Exp, bias=padbias[:])
                    else:
                        nc.scalar.activation(ex[:], ps[:], AF.Exp)
                    expT.append(ex)
                pd = psum("sm", 2, (1, TP))
                for jc in range(2):
                    MM(pd[:], ones_r[:], expT[jc][:], start=(jc == 0), stop=(jc == 1))
                rec = aw.tile([1, TP], f32, tag="rec", name="rec")
                nc.vector.reciprocal(rec[:], pd[:])
                prb = psum("sm", 2, (HD, TP))
                MM(prb[:], one_row[:1, :HD], rec[:1, :], start=True, stop=True)
                rb = aw.tile([HD, TP], f32, tag="rb", name="rb")
                nc.vector.tensor_copy(rb[:], prb[:])
                pa = psum("mm", 4, (HD, TP))
                for jc in range(2):
                    MM(pa[:], v[jc][:, h * HD:(h + 1) * HD], expT[jc][:],
                       start=(jc == 0), stop=(jc == 1))
                at = at1([HD, TP], f32r, f"aT{h}")
                nc.vector.tensor_tensor(at[:], pa[:], rb[:], op=OP.mult)
                aT.append(at)

            wpr, wg = [], []
            for h in range(HEADS):
                wk = at1([HD, D], f32r, f"wproj{h}")
                nc.sync.dma_start(wk[:], wproj_d.ap()[h * HD:(h + 1) * HD])
                wpr.append(wk)
            for kc in range(KC):
                wkg = at1([P, E], f32, f"wg{kc}")
                nc.sync.dma_start(wkg[:], wg_d.ap()[kc * P:(kc + 1) * P])
                wg.append(wkg)

            payload, xn2T = [], []
            for kc in range(KC):
                xn2T.append(at1([P, TP], f32, f"xn2T{kc}"))
            for ti, rows in enumerate(TROWS):
                x2 = at1([P, D], f32, f"x2_{ti}")
                for nf0, nfw in ((0, 512), (512, 256)):
                    pt = psum("mm", 4)
                    for h in range(HEADS):
                        MM(pt[:rows, :nfw], aT[h][:, ti * P:ti * P + rows],
                           wpr[h][:, nf0:nf0 + nfw],
                           start=(h == 0), stop=(h == HEADS - 1))
                    nc.vector.tensor_tensor(x2[:rows, nf0:nf0 + nfw],
                                            pt[:rows, :nfw],
                                            xt[ti][:rows, nf0:nf0 + nfw], op=OP.add)
                pay = at1([P, WIDE], f32, f"pay{ti}")
                nc.vector.memset(pay[:, C_Z2 + 1:WIDE], 0.0)
                payload.append(pay)
                _ln(nc, aw, x2, rows, n2w, n2b, pay[:, C_XN2:C_XN2 + D], eps_t)
                for kc in range(KC):
                    pt = psum("tr", 2, (P, P))
                    nc.tensor.transpose(pt[:, :rows],
                                        pay[:rows, kc * P:(kc + 1) * P],
                                        ident[:rows, :rows])
                    nc.vector.tensor_copy(xn2T[kc][:, ti * P:ti * P + rows],
                                          pt[:, :rows])

            for ti, rows in enumerate(TROWS):
                pay = payload[ti]
                pl = psum("sm", 2, (P, E))
                for kc in range(KC):
                    MM(pl[:rows], xn2T[kc][:, ti * P:ti * P + rows], wg[kc][:],
                       start=(kc == 0), stop=(kc == KC - 1))
                lg = aw.tile([P, E], f32, tag="lg", name="lg")
                nc.vector.tensor_copy(lg[:rows], pl[:rows])
                mx = aw.tile([P, 1], f32, tag="mx", name="mx")
                nc.vector.tensor_reduce(mx[:rows], lg[:rows], axis=X, op=OP.max)
                nmx = aw.tile([P, 1], f32, tag="nmx", name="nmx")
                nc.scalar.mul(nmx[:rows], mx[:rows], -1.0)
                exps = aw.tile([P, E], f32, tag="exps", name="exps")
                se = aw.tile([P, 1], f32, tag="se", name="se")
                nc.scalar.activation(exps[:rows], lg[:rows], AF.Exp,
                                     bias=nmx[:rows], accum_out=se[:rows])
                rc = aw.tile([P, 1], f32, tag="rc", name="rc")
                nc.vector.reciprocal(rc[:rows], se[:rows])
                prbs = pay[:, C_PRB:C_PRB + E]
                nc.vector.tensor_scalar_mul(prbs[:rows], exps[:rows], rc[:rows])
                m8 = aw.tile([P, 8], f32, tag="m8", name="m8")
                nc.vector.max(out=m8[:rows], in_=prbs[:rows])
                msk = aw.tile([P, E], f32, tag="mskp", name="mskp")
                nc.vector.tensor_scalar(msk[:rows], prbs[:rows], m8[:rows, 1:2],
                                        None, op0=OP.is_ge)
                nc.vector.tensor_tensor(pay[:rows, C_CMB:C_CMB + E], prbs[:rows],
                                        msk[:rows], op=OP.mult)
                nc.vector.tensor_scalar(pay[:rows, C_IND:C_IND + E], prbs[:rows],
                                        m8[:rows, 0:1], None, op0=OP.is_equal)
                lse = aw.tile([P, 1], f32, tag="lse", name="lse")
                nc.scalar.activation(lse[:rows], se[:rows], AF.Ln)
                zt = aw.tile([P, 1], f32, tag="zt", name="zt")
                nc.vector.tensor_tensor(zt[:rows], lse[:rows], mx[:rows], op=OP.add)
                nc.scalar.activation(pay[:rows, C_Z2:C_Z2 + 1], zt[:rows], AF.Square)
                nc.sync.dma_start(ag_src[ti * P:ti * P + rows], pay[:rows])

        # ================= AllGather =================
        nc.gpsimd.collective_compute(
            "AllGather", OP.bypass, replica_groups=[CORE_IDS],
            ins=[ag_src[:]], outs=[all_data[0:N]],
        )

        # ================= MoE =================
        with tc.tile_pool(name="mp1", bufs=1) as m1, \
             tc.tile_pool(name="mw", bufs=3) as mw:

            def mt1(shape, dtype, tg):
                return m1.tile(shape, dtype, tag=tg, name=tg)

            psB = tc.alloc_tile_pool(name="psB", bufs=1, space="PSUM")

            def psum(tg, bufs, shape=(P, 512)):
                return psB.tile(list(shape), f32, tag=tg, name=tg, bufs=bufs)

            # ---- aux losses ----
            aux = mt1([P, JN, 17], f32, "aux")
            nc.sync.dma_start(
                aux[:], all_data[0:NPAD, C_IND:C_IND + 17]
                .rearrange("(j p) c -> p j c", p=P))
            ps_s = psum("sm", 2, (17, 1))
            for j in range(JN):
                MM(ps_s[:], aux[:, j, :], ones_f[:], start=(j == 0),
                   stop=(j == JN - 1))
            sums_c = mt1([17, 1], f32, "sums_c")
            nc.vector.tensor_copy(sums_c[:], ps_s[:])
            ps_r = psum("sm", 2, (1, 17))
            MM(ps_r[:], sums_c[:], ident[:17, :17], start=True, stop=True)
            srow = mt1([1, 17], f32, "srow")
            nc.vector.tensor_copy(srow[:], ps_r[:])
            prod = mt1([1, 8], f32, "prod")
            nc.vector.tensor_tensor(prod[:], srow[:1, 0:8], srow[:1, 8:16],
                                    op=OP.mult)
            bal = mt1([1, 1], f32, "bal")
            nc.vector.tensor_reduce(bal[:], prod[:], axis=X, op=OP.add)
            nc.vector.tensor_scalar_mul(bal[:], bal[:], float(E) / (N * N))
            rz = mt1([1, 1], f32, "rz")
            nc.vector.tensor_scalar_mul(rz[:], srow[:1, 16:17], 1.0 / N)
            tot = mt1([1, 1], f32, "tot")
            nc.vector.tensor_scalar_mul(tot[:], bal[:], BALANCE_COEF)
            rzs = mt1([1, 1], f32, "rzs")
            nc.vector.tensor_scalar_mul(rzs[:], rz[:], ROUTER_Z_COEF)
            nc.vector.tensor_tensor(tot[:], tot[:], rzs[:], op=OP.add)
            nc.sync.dma_start(bal_d.ap(), bal[:])
            nc.sync.dma_start(rz_d.ap(), rz[:])
            nc.sync.dma_start(tot_d.ap(), tot[:])

            # ---- gather list for my expert ----
            cmb8 = mt1([P, JN, E], f32, "cmb8")
            nc.sync.dma_start(
                cmb8[:], all_data[0:NPAD, C_CMB:C_CMB + E]
                .rearrange("(j p) c -> p j c", p=P))
            cmbw = mt1([P, JN, E], f32, "cmbw")
            nc.vector.tensor_tensor(cmbw[:], cmb8[:],
                                    esel_bc[:, None, :].to_broadcast([P, JN, E]),
                                    op=OP.mult)
            cmb = mt1([P, JN], f32, "cmb")
            nc.vector.tensor_reduce(cmb[:], cmbw[:], axis=X, op=OP.add)
            msk = mt1([P, JN], f32, "mskm")
            nc.vector.tensor_scalar(msk[:], cmb[:], 0.0, None, op0=OP.is_gt)

            pos_p = psum("sm", 2, (P, JN))
            MM(pos_p[:], LT[:], msk[:], start=True, stop=True)
            cs_p = psum("sm", 2, (JN, 1))
            MM(cs_p[:], msk[:], ones_f[:], start=True, stop=True)
            csum = mt1([JN, 1], f32, "csum")
            nc.vector.tensor_copy(csum[:], cs_p[:])
            cb_p = psum("sm", 2, (JN, 1))
            MM(cb_p[:], LT[:JN, :JN], csum[:], start=True, stop=True)
            cbase = mt1([JN, 1], f32, "cbase")
            nc.vector.tensor_copy(cbase[:], cb_p[:])
            cr_p = psum("sm", 2, (1, JN))
            MM(cr_p[:], cbase[:], ident[:JN, :JN], start=True, stop=True)
            crow = mt1([1, JN], f32, "crow")
            nc.vector.tensor_copy(crow[:], cr_p[:])
            cbb_p = psum("sm", 2, (P, JN))
            MM(cbb_p[:], one_row[:1], crow[:1], start=True, stop=True)
            cbase_bc = mt1([P, JN], f32, "cbase_bc")
            nc.vector.tensor_copy(cbase_bc[:], cbb_p[:])
            pos = mt1([P, JN], f32, "pos")
            nc.vector.tensor_tensor(pos[:], pos_p[:], cbase_bc[:], op=OP.add)
            # slots = mask ? pos : CAP  ==  (pos - CAP)*mask + CAP
            slots = mt1([P, JN], f32, "slots")
            nc.vector.tensor_scalar_add(slots[:], pos[:], -float(CAP))
            nc.vector.tensor_tensor(slots[:], slots[:], msk[:], op=OP.mult)
            nc.vector.tensor_scalar_add(slots[:], slots[:], float(CAP))
            nc.vector.tensor_scalar_min(slots[:], slots[:], float(CAP))
            slots_i = mt1([P, JN], i32, "slots_i")
            nc.vector.tensor_copy(slots_i[:], slots[:])
            ids = mt1([P, JN], i32, "ids")
            nc.gpsimd.iota(ids[:], pattern=[[P, JN]], base=0, channel_multiplier=1)

            glfill = mt1([P, G], i32, "glfill")
            nc.vector.memset(glfill[:], ZROW)
            nc.sync.dma_start(gl[0:CAP].rearrange("(g p) o -> p (g o)", p=P),
                              glfill[:])
            nc.sync.dma_start(gl[CAP:CAP + 1], glfill[:1, :1])
            for j in range(JN):
                nc.gpsimd.indirect_dma_start(
                    out=gl[:], out_offset=bass.IndirectOffsetOnAxis(
                        ap=slots_i[:, j:j + 1], axis=0),
                    in_=ids[:, j:j + 1], in_offset=None)

            # ---- gather tokens + weights; transpose ----
            selT = [mt1([P, CAP], f32r, f"selT{kc}") for kc in range(KC)]
            glts, wslot = [], []
            for g in range(G):
                glt = mt1([P, 1], i32, f"glt{g}")
                nc.sync.dma_start(glt[:], gl[g * P:(g + 1) * P])
                glts.append(glt)
                sel = mt1([P, WIDE], f32, f"sel{g}")
                nc.gpsimd.indirect_dma_start(
                    out=sel[:], out_offset=None,
                    in_=all_data[:],
                    in_offset=bass.IndirectOffsetOnAxis(ap=glt[:, :1], axis=0))
                wtmp = mw.tile([P, E], f32, tag="wtmp", name="wtmp")
                nc.vector.tensor_tensor(wtmp[:], sel[:, C_CMB:C_CMB + E],
                                        esel_bc[:, :], op=OP.mult)
                ws = mt1([P, 1], f32, f"ws{g}")
                nc.vector.tensor_reduce(ws[:], wtmp[:], axis=X, op=OP.add)
                nc.vector.tensor_scalar_mul(ws[:], ws[:], 2.0)  # out = 2*moe_out
                wslot.append(ws)
                for kc in range(KC):
                    pt = psum("tr", 2, (P, P))
                    nc.tensor.transpose(pt[:], sel[:, kc * P:(kc + 1) * P], ident[:])
                    nc.vector.tensor_copy(selT[kc][:, g * P:(g + 1) * P], pt[:])

            # ---- expert MLP: hT ----
            hT = []
            for ht in range(HT):
                w1t = mw.tile([P, KC, P], f32r, tag="w1t", name="w1t")
                nc.sync.dma_start(
                    w1t[:], w1_d.ap()[:, ht * P:(ht + 1) * P]
                    .rearrange("(kc p) f -> p kc f", p=P))
                ph = psum("mm", 4, (P, CAP))
                for kc in range(KC):
                    MM(ph[:], w1t[:, kc, :], selT[kc][:],
                       start=(kc == 0), stop=(kc == KC - 1))
                hh = mt1([P, CAP], f32r, f"hT{ht}")
                nc.scalar.activation(hh[:], ph[:], AF.Gelu, bias=b1_sb[:, ht:ht + 1])
                hT.append(hh)

            # close psB, open psC (8 banks) for the y accumulators
            psB.release()

            with tc.tile_pool(name="psC", bufs=1, space="PSUM") as psC:
                pys = []
                for g in range(G):
                    py1 = psC.tile([P, 512], f32, tag=f"y{g}a", name=f"y{g}a", bufs=1)
                    py2 = psC.tile([P, 512], f32, tag=f"y{g}b", name=f"y{g}b", bufs=1)
                    pys.append((py1, py2))
                for ht in range(HT):
                    w2t = mw.tile([P, D], f32r, tag="w2t", name="w2t")
                    nc.sync.dma_start(w2t[:], w2_d.ap()[ht * P:(ht + 1) * P])
                    for g in range(G):
                        py1, py2 = pys[g]
                        MM(py1[:], hT[ht][:, g * P:(g + 1) * P], w2t[:, 0:512],
                           start=(ht == 0), stop=(ht == HT - 1))
                        MM(py2[:, :256], hT[ht][:, g * P:(g + 1) * P],
                           w2t[:, 512:768], start=(ht == 0), stop=(ht == HT - 1))
                for g in range(G):
                    py1, py2 = pys[g]
                    yg = mt1([P, D], f32, f"yg{g}")
                    nc.vector.tensor_tensor(yg[:, 0:512], py1[:], b2_bc[:, 0:512],
                                            op=OP.add)
                    nc.vector.tensor_tensor(yg[:, 512:768], py2[:, :256],
                                            b2_bc[:, 512:768], op=OP.add)
                    nc.vector.tensor_scalar_mul(yg[:], yg[:], wslot[g][:])
                    nc.gpsimd.indirect_dma_start(
                        out=moe_part[:], out_offset=bass.IndirectOffsetOnAxis(
                            ap=glts[g][:, :1], axis=0),
                        in_=yg[:], in_offset=None)

        # ================= ReduceScatter + output =================
        nc.gpsimd.collective_compute(
            "ReduceScatter", OP.add, replica_groups=[CORE_IDS],
            ins=[moe_part[0:N]], outs=[rs_res[:]],
        )
        nc.sync.dma_start(out_d.ap(), rs_res[0:T])  # rs_res is [TB,D]=[197,D]

        const.release()
        dram.release()

    nc.compile()
    return nc


_NC = None


def _get_nc():
    global _NC
    if _NC is None:
        _NC = build_nc()
    return _NC


def make_in_maps(inputs):
    f = lambda a: np.ascontiguousarray(np.asarray(a, dtype=np.float32))
    in_maps = []
    for c in range(NCORES):
        sel = np.zeros((1, E), np.float32)
        sel[0, c] = 1.0
        in_maps.append({
            "x": f(inputs["x"][c]),
            "Wqkv": f(inputs["Wqkv"]),
            "Wproj": f(inputs["Wproj"]),
            "Wg": f(inputs["Wg"]),
            "norm1_w": f(inputs["norm1_w"]), "norm1_b": f(inputs["norm1_b"]),
            "norm2_w": f(inputs["norm2_w"]), "norm2_b": f(inputs["norm2_b"]),
            "W1": f(inputs["W1"][c]), "b1": f(inputs["b1"][c]),
            "W2": f(inputs["W2"][c]), "b2": f(inputs["b2"][c]),
            "expert_sel": sel,
        })
    return in_maps


def collect_outputs(results):
    out = np.stack([results[c]["out"] for c in range(NCORES)], axis=0)
    r0 = results[0]
    total = np.float32(r0["total_aux"][0, 0])
    bal = np.float32(r0["balance"][0, 0])
    rz = np.float32(r0["router_z"][0, 0])
    return out, total, bal, rz


def kernel(**inputs):
    from concourse.bass_utils import run_bass_kernel_spmd

    nc = _get_nc()
    res = run_bass_kernel_spmd(nc, make_in_maps(inputs), CORE_IDS)
    return collect_outputs(res.results)


# revision 29
# speedup vs baseline: 1.0096x; 1.0096x over previous
"""Trainium2 Bass kernel for nn_BlockMoE (8-core SPMD).

Sharding: data-parallel attention (1 batch element per core), expert-parallel
MoE (1 expert per core) with AllGather token dispatch + indirect-DMA
gather/scatter and ReduceScatter combine. Matmuls run in float32r (fast fp32).

Self-contained: hardcodes shapes B=8, T=197, D=768, E=8, H=3072, heads=12.
"""
import numpy as np

import concourse.bass as bass
import concourse.bacc as bacc
import concourse.mybir as mybir
import concourse.tile as tile
from concourse.masks import make_identity

dt = mybir.dt
AF = mybir.ActivationFunctionType
OP = mybir.AluOpType

# problem shapes
B, T, D = 8, 197, 768
E, H = 8, 3072
HEADS, HD = 12, 64
SCALE = HD ** -0.5
EPS = 1e-5
BALANCE_COEF, ROUTER_Z_COEF = 1e-2, 1e-3

NCORES = 8
CORE_IDS = list(range(NCORES))
P = 128
KC = D // P                      # 6 d-chunks
N = B * T                        # 1576 tokens
JN = 13                          # token chunks of 128 (padded)
NPAD = JN * P                    # 1664
TB = NPAD // NCORES              # 208 rows shipped per core (197 + 11 zero)
ZROW = TB - 1                    # a zero pad row (row 207, core 0 block)
ROWS = NPAD                      # 1664
CAP = 512                        # expert capacity (~394 expected)
G = CAP // P                     # 4 gather tiles
TP = 256                         # padded token free-dim per core
TROWS = (P, T - P)               # (128, 69) token partition chunks
HT = H // P                      # 24

# all_data row layout
C_XN2, C_CMB, C_IND, C_PRB, C_Z2, WIDE = 0, 768, 776, 784, 792, 800

f32, f32r, i32 = dt.float32, dt.float32r, dt.int32
X = mybir.AxisListType.X


def _ln(nc, pool, xt, rows, wrow, brow, out, eps_ap=None):
    """LayerNorm of xt[:rows, :D] -> out[:rows, :D]; wrow/brow [128,D] replicated."""
    s = pool.tile([P, 1], f32, tag="ln_s", name="ln_s")
    nc.vector.tensor_reduce(s[:rows], xt[:rows], axis=X, op=OP.add)
    negmu = pool.tile([P, 1], f32, tag="ln_negmu", name="ln_negmu")
    nc.vector.tensor_scalar_mul(negmu[:rows], s[:rows], -1.0 / D)
    xc = pool.tile([P, D], f32, tag="ln_xc", name="ln_xc")
    nc.vector.tensor_scalar_add(xc[:rows], xt[:rows], negmu[:rows])
    sq = pool.tile([P, D], f32, tag="ln_sq", name="ln_sq")
    var = pool.tile([P, 1], f32, tag="ln_var", name="ln_var")
    nc.scalar.activation(sq[:rows], xc[:rows], AF.Square, accum_out=var[:rows])
    std = pool.tile([P, 1], f32, tag="ln_std", name="ln_std")
    nc.scalar.activation(std[:rows], var[:rows], AF.Sqrt, bias=eps_ap[:rows],
                         scale=1.0 / D)
    rstd = pool.tile([P, 1], f32, tag="ln_rstd", name="ln_rstd")
    nc.vector.reciprocal(rstd[:rows], std[:rows])
    nc.vector.tensor_scalar_mul(xc[:rows], xc[:rows], rstd[:rows])
    nc.vector.tensor_tensor(out[:rows], xc[:rows], wrow[:rows], op=OP.mult)
    nc.vector.tensor_tensor(out[:rows], out[:rows], brow[:rows], op=OP.add)


def build_nc():
    nc = bacc.Bacc("TRN2", target_bir_lowering=False, debug=False,
                   num_devices=NCORES)
    MM = nc.tensor.matmul

    # ---- per-core external I/O ----
    x_d = nc.dram_tensor("x", [T, D], f32, kind="ExternalInput")
    wqkv_d = nc.dram_tensor("Wqkv", [D, 3 * D], f32r, kind="ExternalInput")
    wproj_d = nc.dram_tensor("Wproj", [D, D], f32r, kind="ExternalInput")
    wg_d = nc.dram_tensor("Wg", [D, E], f32, kind="ExternalInput")
    n1w_d = nc.dram_tensor("norm1_w", [D], f32, kind="ExternalInput")
    n1b_d = nc.dram_tensor("norm1_b", [D], f32, kind="ExternalInput")
    n2w_d = nc.dram_tensor("norm2_w", [D], f32, kind="ExternalInput")
    n2b_d = nc.dram_tensor("norm2_b", [D], f32, kind="ExternalInput")
    w1_d = nc.dram_tensor("W1", [D, H], f32r, kind="ExternalInput")
    b1_d = nc.dram_tensor("b1", [H], f32, kind="ExternalInput")
    w2_d = nc.dram_tensor("W2", [H, D], f32r, kind="ExternalInput")
    b2_d = nc.dram_tensor("b2", [D], f32, kind="ExternalInput")
    esel_d = nc.dram_tensor("expert_sel", [1, E], f32, kind="ExternalInput")

    out_d = nc.dram_tensor("out", [T, D], f32, kind="ExternalOutput")
    bal_d = nc.dram_tensor("balance", [1, 1], f32, kind="ExternalOutput")
    rz_d = nc.dram_tensor("router_z", [1, 1], f32, kind="ExternalOutput")
    tot_d = nc.dram_tensor("total_aux", [1, 1], f32, kind="ExternalOutput")

    with tile.TileContext(nc) as tc:
        dram = tc.alloc_tile_pool(name="dram", bufs=1, space="DRAM")
        ag_src = dram.tile([TB, WIDE], f32, tag="ag_src", name="ag_src")
        all_data = dram.tile([ROWS, WIDE], f32, tag="all_data", name="all_data")
        gl = dram.tile([CAP + 1, 1], i32, tag="gl", name="gl")
        moe_part = dram.tile([ROWS, D], f32, tag="moe_part", name="moe_part")
        rs_res = dram.tile([TB, D], f32, tag="rs_res", name="rs_res")

        const = tc.alloc_tile_pool(name="const", bufs=1)

        def ct(shape, dtype, tg):
            return const.tile(shape, dtype, tag=tg, name=tg)

        ident = ct([P, P], f32, "ident")
        make_identity(nc, ident)
        ones_f = ct([P, 1], f32, "ones_f")
        nc.vector.memset(ones_f[:], 1.0)
        ones_r = ct([P, 1], f32r, "ones_r")
        nc.vector.tensor_copy(ones_r[:], ones_f[:])
        one_row = ct([1, P], f32, "one_row")
        nc.vector.memset(one_row[:], 1.0)
        LT = ct([P, P], f32, "LT")  # LT[p, m] = 1 if p < m
        nc.gpsimd.memset(LT[:], 1.0)
        nc.gpsimd.affine_select(out=LT[:], in_=LT[:], compare_op=OP.is_gt,
                                fill=0.0, base=0, pattern=[[1, P]],
                                channel_multiplier=-1)
        zrow = ct([P, WIDE], f32, "zrow")
        nc.vector.memset(zrow[:], 0.0)
        eps_t = ct([P, 1], f32, "eps_t")
        nc.vector.memset(eps_t[:], EPS)
        # padbias[p] = 0 for p < 69 (valid tail tokens), -1e30 beyond -> exp()=0
        padbias = ct([P, 1], f32, "padbias")
        nc.gpsimd.memset(padbias[:], 0.0)
        nc.gpsimd.affine_select(out=padbias[:], in_=padbias[:],
                                compare_op=OP.is_ge, fill=-1e30,
                                base=TROWS[1] - 1, pattern=[[1, 1]],
                                channel_multiplier=-1)

        nrm_bc = ct([P, 4 * D], f32, "nrm_bc")
        b2_bc = ct([P, D], f32, "b2_bc")
        esel_bc = ct([P, E], f32, "esel_bc")
        b1_sb = ct([P, HT], f32, "b1_sb")
        nc.sync.dma_start(b1_sb[:], b1_d.ap().rearrange("(o p) -> p o", p=P))

        # broadcast rows via K=1 matmul
        with tc.tile_pool(name="bcld", bufs=1) as blp, \
             tc.tile_pool(name="bcps", bufs=2, space="PSUM") as bpp:
            nrm = blp.tile([1, 4 * D], f32, tag="nrm", name="nrm")
            nc.sync.dma_start(nrm[:1, 0:D], n1w_d.ap()[None, :])
            nc.sync.dma_start(nrm[:1, D:2 * D], n1b_d.ap()[None, :])
            nc.sync.dma_start(nrm[:1, 2 * D:3 * D], n2w_d.ap()[None, :])
            nc.sync.dma_start(nrm[:1, 3 * D:4 * D], n2b_d.ap()[None, :])
            b2r = blp.tile([1, D], f32, tag="b2r", name="b2r")
            nc.sync.dma_start(b2r[:1, :], b2_d.ap()[None, :])
            eselr = blp.tile([1, E], f32, tag="eselr", name="eselr")
            nc.sync.dma_start(eselr[:1, :], esel_d.ap()[:1, :])
            for src, dst, width in ((nrm, nrm_bc, 4 * D), (b2r, b2_bc, D),
                                    (eselr, esel_bc, E)):
                done = 0
                while done < width:
                    w = min(512, width - done)
                    pt = bpp.tile([P, 512], f32, tag="bc", name="bc_ps")
                    MM(pt[:, :w], one_row[:1], src[:1, done:done + w],
                       start=True, stop=True)
                    nc.vector.tensor_copy(dst[:, done:done + w], pt[:, :w])
                    done += w

        n1w, n1b = nrm_bc[:, 0:D], nrm_bc[:, D:2 * D]
        n2w, n2b = nrm_bc[:, 2 * D:3 * D], nrm_bc[:, 3 * D:4 * D]

        # zero pad rows of ag_src (rows T..TB) and all of moe_part (early)
        nc.sync.dma_start(ag_src[T:TB], zrow[:TB - T])
        for j in range(JN):
            nc.sync.dma_start(moe_part[j * P:(j + 1) * P], zrow[:, :D])


        # ================= attention (own batch) =================
        with tc.tile_pool(name="ap1", bufs=1) as a1, \
             tc.tile_pool(name="aw", bufs=2) as aw, \
             tc.tile_pool(name="psA", bufs=1, space="PSUM") as psA:

            def at1(shape, dtype, tg):
                return a1.tile(shape, dtype, tag=tg, name=tg)

            def psum(tg, bufs, shape=(P, 512)):
                return psA.tile(list(shape), f32, tag=tg, name=tg, bufs=bufs)

            xt, xn = [], []
            for ti, rows in enumerate(TROWS):
                xi = at1([P, D], f32, f"x{ti}")
                nc.sync.dma_start(xi[:rows], x_d.ap()[ti * P:ti * P + rows])
                xt.append(xi)
                xni = at1([P, D], f32, f"xn{ti}")
                _ln(nc, aw, xi, rows, n1w, n1b, xni, eps_t)
                xn.append(xni)

            # xnT [6][128, TP] f32r, zero-padded cols
            xnT = []
            for kc in range(KC):
                xTk = at1([P, TP], f32r, f"xnT{kc}")
                nc.vector.tensor_copy(xTk[:, T:], zrow[:, :TP - T])
                for ti, rows in enumerate(TROWS):
                    pt = psum("tr", 2, (P, P))
                    nc.tensor.transpose(pt[:, :rows],
                                        xn[ti][:rows, kc * P:(kc + 1) * P],
                                        ident[:rows, :rows])
                    nc.vector.tensor_copy(xTk[:, ti * P:ti * P + rows], pt[:, :rows])
                xnT.append(xTk)

            wqkv = []
            for kc in range(KC):
                wk = at1([P, 3 * D], f32r, f"wqkv{kc}")
                nc.sync.dma_start(wk[:], wqkv_d.ap()[kc * P:(kc + 1) * P])
                wqkv.append(wk)

            qT, kT = [], []
            for jt in range(12):
                pt = psum("mm", 4, (P, TP))
                for kc in range(KC):
                    MM(pt[:], wqkv[kc][:, jt * P:(jt + 1) * P], xnT[kc][:],
                       start=(kc == 0), stop=(kc == KC - 1))
                o = at1([P, TP], f32r, f"qkT{jt}")
                if jt < 6:
                    nc.vector.tensor_scalar_mul(o[:], pt[:], SCALE)
                    qT.append(o)
                else:
                    nc.vector.tensor_copy(o[:], pt[:])
                    kT.append(o)

            v = []
            for ti in range(2):
                vt = at1([P, D], f32r, f"v{ti}")
                for nf0, nfw in ((0, 512), (512, 256)):
                    pt = psum("mm", 4)
                    for kc in range(KC):
                        MM(pt[:, :nfw], xnT[kc][:, ti * P:(ti + 1) * P],
                           wqkv[kc][:, 2 * D + nf0:2 * D + nf0 + nfw],
                           start=(kc == 0), stop=(kc == KC - 1))
                    nc.vector.tensor_copy(vt[:, nf0:nf0 + nfw], pt[:, :nfw])
                v.append(vt)

            aT = []
            for h in range(HEADS):
                jk, po = h // 2, (h % 2) * HD
                expT = []
                for jc in range(2):
                    ps = psum("mm", 4, (P, TP))
                    MM(ps[:], kT[jk][po:po + HD, jc * P:(jc + 1) * P],
                       qT[jk][po:po + HD, :], start=True, stop=True)
                    ex = aw.tile([P, TP], f32r, tag=f"expT{jc}", name=f"expT{jc}")
                    if jc == 1:
                        nc.scalar.activation(ex[:], ps[:], AF.Exp, bias=padbias[:])
                    else:
                        nc.scalar.activation(ex[:], ps[:], AF.Exp)
                    expT.append(ex)
                pd = psum("sm", 2, (1, TP))
                for jc in range(2):
                    MM(pd[:], ones_r[:], expT[jc][:], start=(jc == 0), stop=(jc == 1))
                rec = aw.tile([1, TP], f32, tag="rec", name="rec")
                nc.vector.reciprocal(rec[:], pd[:])
                prb = psum("sm", 2, (HD, TP))
                MM(prb[:], one_row[:1, :HD], rec[:1, :], start=True, stop=True)
                rb = aw.tile([HD, TP], f32, tag="rb", name="rb")
                nc.vector.tensor_copy(rb[:], prb[:])
                pa = psum("mm", 4, (HD, TP))
                for jc in range(2):
                    MM(pa[:], v[jc][:, h * HD:(h + 1) * HD], expT[jc][:],
                       start=(jc == 0), stop=(jc == 1))
                at = at1([HD, TP], f32r, f"aT{h}")
                nc.vector.tensor_tensor(at[:], pa[:], rb[:], op=OP.mult)
                aT.append(at)

            wpr, wg = [], []
            for h in range(HEADS):
                wk = at1([HD, D], f32r, f"wproj{h}")
                nc.sync.dma_start(wk[:], wproj_d.ap()[h * HD:(h + 1) * HD])
                wpr.append(wk)
            for kc in range(KC):
                wkg = at1([P, E], f32, f"wg{kc}")
                nc.sync.dma_start(wkg[:], wg_d.ap()[kc * P:(kc + 1) * P])
                wg.append(wkg)

            payload, xn2T = [], []
            for kc in range(KC):
                xn2T.append(at1([P, TP], f32, f"xn2T{kc}"))
            for ti, rows in enumerate(TROWS):
                x2 = at1([P, D], f32, f"x2_{ti}")
                for nf0, nfw in ((0, 512), (512, 256)):
                    pt = psum("mm", 4)
                    for h in range(HEADS):
                        MM(pt[:rows, :nfw], aT[h][:, ti * P:ti * P + rows],
                           wpr[h][:, nf0:nf0 + nfw],
                           start=(h == 0), stop=(h == HEADS - 1))
                    nc.vector.tensor_tensor(x2[:rows, nf0:nf0 + nfw],
                                            pt[:rows, :nfw],
                                            xt[ti][:rows, nf0:nf0 + nfw], op=OP.add)
                pay = at1([P, WIDE], f32, f"pay{ti}")
                nc.vector.memset(pay[:, C_Z2 + 1:WIDE], 0.0)
                payload.append(pay)
                _ln(nc, aw, x2, rows, n2w, n2b, pay[:, C_XN2:C_XN2 + D], eps_t)
                for kc in range(KC):
                    pt = psum("tr", 2, (P, P))
                    nc.tensor.transpose(pt[:, :rows],
                                        pay[:rows, kc * P:(kc + 1) * P],
                                        ident[:rows, :rows])
                    nc.vector.tensor_copy(xn2T[kc][:, ti * P:ti * P + rows],
                                          pt[:, :rows])

            for ti, rows in enumerate(TROWS):
                pay = payload[ti]
                pl = psum("sm", 2, (P, E))
                for kc in range(KC):
                    MM(pl[:rows], xn2T[kc][:, ti * P:ti * P + rows], wg[kc][:],
                       start=(kc == 0), stop=(kc == KC - 1))
                lg = aw.tile([P, E], f32, tag="lg", name="lg")
                nc.vector.tensor_copy(lg[:rows], pl[:rows])
                mx = aw.tile([P, 1], f32, tag="mx", name="mx")
                nc.vector.tensor_reduce(mx[:rows], lg[:rows], axis=X, op=OP.max)
                nmx = aw.tile([P, 1], f32, tag="nmx", name="nmx")
                nc.scalar.mul(nmx[:rows], mx[:rows], -1.0)
                exps = aw.tile([P, E], f32, tag="exps", name="exps")
                se = aw.tile([P, 1], f32, tag="se", name="se")
                nc.scalar.activation(exps[:rows], lg[:rows], AF.Exp,
                                     bias=nmx[:rows], accum_out=se[:rows])
                rc = aw.tile([P, 1], f32, tag="rc", name="rc")
                nc.vector.reciprocal(rc[:rows], se[:rows])
                prbs = pay[:, C_PRB:C_PRB + E]
                nc.vector.tensor_scalar_mul(prbs[:rows], exps[:rows], rc[:rows])
                m8 = aw.tile([P, 8], f32, tag="m8", name="m8")
                nc.vector.max(out=m8[:rows], in_=prbs[:rows])
                msk = aw.tile([P, E], f32, tag="mskp", name="mskp")
                nc.vector.tensor_scalar(msk[:rows], prbs[:rows], m8[:rows, 1:2],
                                        None, op0=OP.is_ge)
                nc.vector.tensor_tensor(pay[:rows, C_CMB:C_CMB + E], prbs[:rows],
                                        msk[:rows], op=OP.mult)
                nc.vector.tensor_scalar(pay[:rows, C_IND:C_IND + E], prbs[:rows],
                                        m8[:rows, 0:1], None, op0=OP.is_equal)
                lse = aw.tile([P, 1], f32, tag="lse", name="lse")
                nc.scalar.activation(lse[:rows], se[:rows], AF.Ln)
                zt = aw.tile([P, 1], f32, tag="zt", name="zt")
                nc.vector.tensor_tensor(zt[:rows], lse[:rows], mx[:rows], op=OP.add)
                nc.scalar.activation(pay[:rows, C_Z2:C_Z2 + 1], zt[:rows], AF.Square)
                nc.sync.dma_start(ag_src[ti * P:ti * P + rows], pay[:rows])

        # ================= AllGather =================
        nc.gpsimd.collective_compute(
            "AllGather", OP.bypass, replica_groups=[CORE_IDS],
            ins=[ag_src[:]], outs=[all_data[:]],
        )

        # ================= MoE =================
        with tc.tile_pool(name="mp1", bufs=1) as m1, \
             tc.tile_pool(name="mw", bufs=3) as mw:

            def mt1(shape, dtype, tg):
                return m1.tile(shape, dtype, tag=tg, name=tg)

            psB = tc.alloc_tile_pool(name="psB", bufs=1, space="PSUM")

            def psum(tg, bufs, shape=(P, 512)):
                return psB.tile(list(shape), f32, tag=tg, name=tg, bufs=bufs)

            # ---- aux losses ----
            aux = mt1([P, JN, 17], f32, "aux")
            nc.sync.dma_start(
                aux[:], all_data[0:NPAD, C_IND:C_IND + 17]
                .rearrange("(j p) c -> p j c", p=P))
            ps_s = psum("sm", 2, (17, 1))
            for j in range(JN):
                MM(ps_s[:], aux[:, j, :], ones_f[:], start=(j == 0),
                   stop=(j == JN - 1))
            sums_c = mt1([17, 1], f32, "sums_c")
            nc.vector.tensor_copy(sums_c[:], ps_s[:])
            ps_r = psum("sm", 2, (1, 17))
            MM(ps_r[:], sums_c[:], ident[:17, :17], start=True, stop=True)
            srow = mt1([1, 17], f32, "srow")
            nc.vector.tensor_copy(srow[:], ps_r[:])
            prod = mt1([1, 8], f32, "prod")
            nc.vector.tensor_tensor(prod[:], srow[:1, 0:8], srow[:1, 8:16],
                                    op=OP.mult)
            bal = mt1([1, 1], f32, "bal")
            nc.vector.tensor_reduce(bal[:], prod[:], axis=X, op=OP.add)
            nc.vector.tensor_scalar_mul(bal[:], bal[:], float(E) / (N * N))
            rz = mt1([1, 1], f32, "rz")
            nc.vector.tensor_scalar_mul(rz[:], srow[:1, 16:17], 1.0 / N)
            tot = mt1([1, 1], f32, "tot")
            nc.vector.tensor_scalar_mul(tot[:], bal[:], BALANCE_COEF)
            rzs = mt1([1, 1], f32, "rzs")
            nc.vector.tensor_scalar_mul(rzs[:], rz[:], ROUTER_Z_COEF)
            nc.vector.tensor_tensor(tot[:], tot[:], rzs[:], op=OP.add)
            nc.sync.dma_start(bal_d.ap(), bal[:])
            nc.sync.dma_start(rz_d.ap(), rz[:])
            nc.sync.dma_start(tot_d.ap(), tot[:])

            # ---- gather list for my expert ----
            cmb8 = mt1([P, JN, E], f32, "cmb8")
            nc.sync.dma_start(
                cmb8[:], all_data[0:NPAD, C_CMB:C_CMB + E]
                .rearrange("(j p) c -> p j c", p=P))
            cmbw = mt1([P, JN, E], f32, "cmbw")
            nc.vector.tensor_tensor(cmbw[:], cmb8[:],
                                    esel_bc[:, None, :].to_broadcast([P, JN, E]),
                                    op=OP.mult)
            cmb = mt1([P, JN], f32, "cmb")
            nc.vector.tensor_reduce(cmb[:], cmbw[:], axis=X, op=OP.add)
            msk = mt1([P, JN], f32, "mskm")
            nc.vector.tensor_scalar(msk[:], cmb[:], 0.0, None, op0=OP.is_gt)

            pos_p = psum("sm", 2, (P, JN))
            MM(pos_p[:], LT[:], msk[:], start=True, stop=True)
            cs_p = psum("sm", 2, (JN, 1))
            MM(cs_p[:], msk[:], ones_f[:], start=True, stop=True)
            csum = mt1([JN, 1], f32, "csum")
            nc.vector.tensor_copy(csum[:], cs_p[:])
            cb_p = psum("sm", 2, (JN, 1))
            MM(cb_p[:], LT[:JN, :JN], csum[:], start=True, stop=True)
            cbase = mt1([JN, 1], f32, "cbase")
            nc.vector.tensor_copy(cbase[:], cb_p[:])
            cr_p = psum("sm", 2, (1, JN))
            MM(cr_p[:], cbase[:], ident[:JN, :JN], start=True, stop=True)
            crow = mt1([1, JN], f32, "crow")
            nc.vector.tensor_copy(crow[:], cr_p[:])
            cbb_p = psum("sm", 2, (P, JN))
            MM(cbb_p[:], one_row[:1], crow[:1], start=True, stop=True)
            cbase_bc = mt1([P, JN], f32, "cbase_bc")
            nc.vector.tensor_copy(cbase_bc[:], cbb_p[:])
            pos = mt1([P, JN], f32, "pos")
            nc.vector.tensor_tensor(pos[:], pos_p[:], cbase_bc[:], op=OP.add)
            # slots = mask ? pos : CAP  ==  (pos - CAP)*mask + CAP
            slots = mt1([P, JN], f32, "slots")
            nc.vector.tensor_scalar_add(slots[:], pos[:], -float(CAP))
            nc.vector.tensor_tensor(slots[:], slots[:], msk[:], op=OP.mult)
            nc.vector.tensor_scalar_add(slots[:], slots[:], float(CAP))
            nc.vector.tensor_scalar_min(slots[:], slots[:], float(CAP))
            slots_i = mt1([P, JN], i32, "slots_i")
            nc.vector.tensor_copy(slots_i[:], slots[:])
            ids = mt1([P, JN], i32, "ids")
            nc.gpsimd.iota(ids[:], pattern=[[P, JN]], base=0, channel_multiplier=1)

            glfill = mt1([P, G], i32, "glfill")
            nc.vector.memset(glfill[:], ZROW)
            nc.sync.dma_start(gl[0:CAP].rearrange("(g p) o -> p (g o)", p=P),
                              glfill[:])
            nc.sync.dma_start(gl[CAP:CAP + 1], glfill[:1, :1])
            for j in range(JN):
                nc.gpsimd.indirect_dma_start(
                    out=gl[:], out_offset=bass.IndirectOffsetOnAxis(
                        ap=slots_i[:, j:j + 1], axis=0),
                    in_=ids[:, j:j + 1], in_offset=None)

            # ---- gather tokens + weights; transpose ----
            selT = [mt1([P, CAP], f32r, f"selT{kc}") for kc in range(KC)]
            glts, wslot = [], []
            for g in range(G):
                glt = mt1([P, 1], i32, f"glt{g}")
                nc.sync.dma_start(glt[:], gl[g * P:(g + 1) * P])
                glts.append(glt)
                sel = mt1([P, WIDE], f32, f"sel{g}")
                nc.gpsimd.indirect_dma_start(
                    out=sel[:], out_offset=None,
                    in_=all_data[:],
                    in_offset=bass.IndirectOffsetOnAxis(ap=glt[:, :1], axis=0))
                wtmp = mw.tile([P, E], f32, tag="wtmp", name="wtmp")
                nc.vector.tensor_tensor(wtmp[:], sel[:, C_CMB:C_CMB + E],
                                        esel_bc[:, :], op=OP.mult)
                ws = mt1([P, 1], f32, f"ws{g}")
                nc.vector.tensor_reduce(ws[:], wtmp[:], axis=X, op=OP.add)
                nc.vector.tensor_scalar_mul(ws[:], ws[:], 2.0)  # out = 2*moe_out
                wslot.append(ws)
                for kc in range(KC):
                    pt = psum("tr", 2, (P, P))
                    nc.tensor.transpose(pt[:], sel[:, kc * P:(kc + 1) * P], ident[:])
                    nc.vector.tensor_copy(selT[kc][:, g * P:(g + 1) * P], pt[:])

            # ---- expert MLP: hT ----
            hT = []
            for ht in range(HT):
                w1t = mw.tile([P, KC, P], f32r, tag="w1t", name="w1t")
                nc.sync.dma_start(
                    w1t[:], w1_d.ap()[:, ht * P:(ht + 1) * P]
                    .rearrange("(kc p) f -> p kc f", p=P))
                ph = psum("mm", 4, (P, CAP))
                for kc in range(KC):
                    MM(ph[:], w1t[:, kc, :], selT[kc][:],
                       start=(kc == 0), stop=(kc == KC - 1))
                hh = mt1([P, CAP], f32r, f"hT{ht}")
                nc.scalar.activation(hh[:], ph[:], AF.Gelu, bias=b1_sb[:, ht:ht + 1])
                hT.append(hh)

            # close psB, open psC (8 banks) for the y accumulators
            psB.release()

            with tc.tile_pool(name="psC", bufs=1, space="PSUM") as psC:
                pys = []
                for g in range(G):
                    py1 = psC.tile([P, 512], f32, tag=f"y{g}a", name=f"y{g}a", bufs=1)
                    py2 = psC.tile([P, 512], f32, tag=f"y{g}b", name=f"y{g}b", bufs=1)
                    pys.append((py1, py2))
                for ht in range(HT):
                    w2t = mw.tile([P, D], f32r, tag="w2t", name="w2t")
                    nc.sync.dma_start(w2t[:], w2_d.ap()[ht * P:(ht + 1) * P])
                    for g in range(G):
                        py1, py2 = pys[g]
                        MM(py1[:], hT[ht][:, g * P:(g + 1) * P], w2t[:, 0:512],
                           start=(ht == 0), stop=(ht == HT - 1))
                        MM(py2[:, :256], hT[ht][:, g * P:(g + 1) * P],
                           w2t[:, 512:768], start=(ht == 0), stop=(ht == HT - 1))
                for g in range(G):
                    py1, py2 = pys[g]
                    yg = mt1([P, D], f32, f"yg{g}")
                    nc.vector.tensor_tensor(yg[:, 0:512], py1[:], b2_bc[:, 0:512],
                                            op=OP.add)
                    nc.vector.tensor_tensor(yg[:, 512:768], py2[:, :256],
                                            b2_bc[:, 512:768], op=OP.add)
                    nc.vector.tensor_scalar_mul(yg[:], yg[:], wslot[g][:])
                    nc.gpsimd.indirect_dma_start(
                        out=moe_part[:], out_offset=bass.IndirectOffsetOnAxis(
                            ap=glts[g][:, :1], axis=0),
                        in_=yg[:], in_offset=None)

        # ================= ReduceScatter + output =================
        nc.gpsimd.collective_compute(
            "ReduceScatter", OP.add, replica_groups=[CORE_IDS],
            ins=[moe_part[:]], outs=[rs_res[:]],
        )
        nc.sync.dma_start(out_d.ap(), rs_res[0:T])

        const.release()
        dram.release()

    nc.compile()
    return nc


_NC = None


def _get_nc():
    global _NC
    if _NC is None:
        _NC = build_nc()
    return _NC


def make_in_maps(inputs):
    f = lambda a: np.ascontiguousarray(np.asarray(a, dtype=np.float32))
    in_maps = []
    for c in range(NCORES):
        sel = np.zeros((1, E), np.float32)
        sel[0, c] = 1.0
        in_maps.append({
            "x": f(inputs["x"][c]),
            "Wqkv": f(inputs["Wqkv"]),
            "Wproj": f(inputs["Wproj"]),
            "Wg": f(inputs["Wg"]),
            "norm1_w": f(inputs["norm1_w"]), "norm1_b": f(inputs["norm1_b"]),
            "norm2_w": f(inputs["norm2_w"]), "norm2_b": f(inputs["norm2_b"]),
            "W1": f(inputs["W1"][c]), "b1": f(inputs["b1"][c]),
            "W2": f(inputs["W2"][c]), "b2": f(inputs["b2"][c]),
            "expert_sel": sel,
        })
    return in_maps


def collect_outputs(results):
    out = np.stack([results[c]["out"] for c in range(NCORES)], axis=0)
    r0 = results[0]
    total = np.float32(r0["total_aux"][0, 0])
    bal = np.float32(r0["balance"][0, 0])
    rz = np.float32(r0["router_z"][0, 0])
    return out, total, bal, rz


def kernel(**inputs):
    from concourse.bass_utils import run_bass_kernel_spmd

    nc = _get_nc()
    res = run_bass_kernel_spmd(nc, make_in_maps(inputs), CORE_IDS)
    return collect_outputs(res.results)
